# revision 28
# baseline (speedup 1.0000x reference)
"""EntropyBottleneck forward kernel for Trainium2 (8 NeuronCores, data-parallel).

Math: with the per-channel gate params f == 0 (always true for this problem's
inputs), each _logits_cumulative layer is affine, so the whole 4-layer chain
collapses to t = a_c * x + d_c per channel c (a_c ~= 0.125). The likelihood is

    lik = | sigmoid(s*(t+h)) - sigmoid(s*(t-h)) |,  s = -sign(2t), h = a_c/2
        =   sigmoid(t+h) - sigmoid(t-h)            (identical in exact math)

The sign trick in the reference only guards against fp32 cancellation when
both sigmoids saturate; here |t| <= ~4.5 (a ~= 0.125, |o| <= ~25, |d| <= ~1.5)
so sigma ranges over ~[0.01, 0.99] and the direct fp32 difference is accurate.
lik in [2.7e-3, h/2], so the 1e-9 low_bound clip never binds and lik fits a
uint8 fixed-point format exactly.

Layout: the host packs each core's [62500, 64] slab as a transposed
[128, 31250] array, which puts the channel index on the SBUF partition axis
for free (partition p holds channel p % 64). All per-channel params are then
plain per-partition [128,1] scale/bias vectors: no TensorE transposes and no
PSUM use at all. Device work per element: o = x + n (DVE), two sigmoids (ACT
with per-partition scale a and biases d+-h, fp32 internally), subtract +
uint8 quantize (DVE). All compute hides behind the DMAs.

The kernel is DMA-bound, so I/O dtypes are chosen as small as the 2e-2
norm-relative-error gate safely allows (measured ~1.7e-3): x fp16, n fp8
(e4m3), o fp16, lik uint8 (scale 8064 ~= 2*127/h; values <= 252). That is
24 MB/core per invocation vs 64 MB for fp32. DMA uses both HWDGE rings (SP:
x-load + a sliver of n; ACT: o-store + a sliver of lik) plus SWDGE (gpsimd)
for ~92% of the 1-byte n/lik streams, byte-balanced across the three queues
("bal3"); 4096-column tiles give 8 KB (4 KB) per-partition segments, the
measured DMA sweet spot. Aggregate achieved ~320 GB/s/core; pure-DMA
ablations put the machine ceiling at ~330-340 GB/s/core.

Sharding: data-parallel over points N across the 8 cores; tiny params
replicated; no cross-core communication.
"""

import numpy as np

N_TOTAL = 500000
C = 64
N_CORES = 8
ROWS_PER_CORE = N_TOTAL // N_CORES          # 62500
ELEMS = ROWS_PER_CORE * C                   # 4,000,000 per core
CHUNKS = ELEMS // 128                       # 31250 = columns of the [128, COLS] view
COLS = CHUNKS
TILE_F = 4096                               # main tile width (8 KB/partition fp16)
N_FULL_TILES = COLS // TILE_F               # 15
TAIL_F = COLS - N_FULL_TILES * TILE_F       # 530

VARIANT = "sig2"        # "sig2" = exact two-sigmoid; "tanh" = 2h*sigma'(t)
RING = "bal3"
N_DTYPE = "f8"          # "f16" | "f8" (float8 e4m3 noise)
LIK_DTYPE = "u8"        # "f16" | "i8" | "u8" (int likelihood, scale 1/LIK_SCALE)
LIK_SCALES = {"i8": 4032.0, "u8": 8064.0}   # lik <= h/2 ~= 0.03125
COLS_PAD = COLS         # dram row pitch; 32768 aligns partition stride to 64KB

_CACHE: dict = {}


def _softplus64(x):
    return np.log1p(np.exp(-np.abs(x))) + np.maximum(x, 0.0)


def _collapse_affine(inputs):
    """Fold the 4 affine layers into per-channel (a, d) in float64."""
    alpha = None
    beta = None
    for i in range(4):
        W = _softplus64(np.asarray(inputs[f"m{i}"], dtype=np.float64))  # (C, fo, fi)
        bb = np.asarray(inputs[f"b{i}"], dtype=np.float64)[:, :, 0]     # (C, fo)
        if i == 0:
            alpha = W[:, :, 0]
            beta = bb
        else:
            alpha = np.einsum("cij,cj->ci", W, alpha)
            beta = np.einsum("cij,cj->ci", W, beta) + bb
    return alpha[:, 0], beta[:, 0]  # (C,), (C,)


def _build_bass(reps=1, variant=None, ring_mode=None, stage=4, sub_eng="vector",
                tile_f=None, n_dtype=None, lik_dtype=None, cols_pad=None,
                bal_frac=None, io_bufs=4, **_legacy):
    # stage: 0 = pure DMA passthrough (o <- x, lik <- n), 4 = full kernel
    if variant is None:
        variant = VARIANT
    if ring_mode is None:
        ring_mode = RING
    if tile_f is None:
        tile_f = TILE_F
    if n_dtype is None:
        n_dtype = N_DTYPE
    if lik_dtype is None:
        lik_dtype = LIK_DTYPE
    if cols_pad is None:
        cols_pad = COLS_PAD
    n_full, tail_f = divmod(COLS, tile_f)
    import concourse.bacc as bacc
    import concourse.mybir as mybir
    from concourse.mybir import ActivationFunctionType as AF
    from concourse.mybir import AluOpType as ALU
    from concourse.tile import TileContext

    f16 = mybir.dt.float16
    f32 = mybir.dt.float32
    nc = bacc.Bacc("TRN2", target_bir_lowering=False, debug=False,
                   enable_asserts=False, num_devices=N_CORES)

    f8 = mybir.dt.float8e4
    ndt = f8 if n_dtype == "f8" else f16
    ldt = {"i8": mybir.dt.int8, "u8": mybir.dt.uint8, "f16": f16}[lik_dtype]
    assert stage != 0 or (n_dtype == "f16" and lik_dtype == "f16")

    CP = cols_pad
    x_d = nc.dram_tensor("x", [128, CP], f16, kind="ExternalInput")
    n_d = nc.dram_tensor("n", [128, CP], ndt, kind="ExternalInput")
    prm_d = nc.dram_tensor("prm", [128, 8], f32, kind="ExternalInput")
    o_d = nc.dram_tensor("o", [128, CP], f16, kind="ExternalOutput")
    lik_d = nc.dram_tensor("lik", [128, CP], ldt, kind="ExternalOutput")

    with TileContext(nc) as tc:
        with (
            tc.tile_pool(name="const", bufs=1) as constp,
            tc.tile_pool(name="io", bufs=io_bufs) as iop,
            tc.tile_pool(name="work", bufs=2) as workp,
        ):
            prm = constp.tile([128, 8], f32)
            nc.sync.dma_start(prm[:], prm_d[:, :])
            a_ap = prm[:, 0:1]       # a
            bu_ap = prm[:, 1:2]      # d + h
            bl_ap = prm[:, 2:3]      # d - h
            a2_ap = prm[:, 3:4]      # a / 2
            d2_ap = prm[:, 4:5]      # d / 2
            h2_ap = prm[:, 5:6]      # h / 2
            nh2_ap = prm[:, 6:7]     # -h / 2

            # engine per DMA stream: (x-load, n-load, o-store, lik-store);
            # "bal3" splits n-load/lik-store between a HW ring and SWDGE so
            # bytes balance across SP/ACT/SWDGE in proportion to their rates.
            bal3 = ring_mode == "bal3"
            if ring_mode == "sw7":
                engs = (nc.sync, nc.gpsimd, nc.scalar, nc.gpsimd)
            elif ring_mode == "sw2":
                engs = (nc.sync, nc.gpsimd, nc.scalar, nc.sync)
            elif ring_mode in ("ls", "bal3", "balt"):
                engs = (nc.sync, nc.sync, nc.scalar, nc.scalar)
            elif ring_mode == "sw0":
                engs = (nc.sync, nc.gpsimd, nc.scalar, nc.gpsimd)
            elif ring_mode == "swx":
                engs = (nc.gpsimd, nc.sync, nc.scalar, nc.gpsimd)
            else:
                raise ValueError(ring_mode)
            ld_x, ld_n, st_o, st_l = engs

            sub = {"vector": nc.vector, "gpsimd": nc.gpsimd}[sub_eng]

            # bal3 split: fraction `frac` of the n-load (and lik-store) rides
            # the HW ring with x (o), the rest goes to SWDGE, chosen so that
            # ring byte-time balances at rates SP=ACT=157, SWDGE=140 GB/s:
            #   (x_b + frac*n_b)/157 = (1-frac)*(n_b + l_b)/140
            x_b = 2.0
            n_b = 1.0 if n_dtype == "f8" else 2.0
            l_b = 1.0 if lik_dtype in ("i8", "u8") else 2.0
            s_b = 0.5 * (n_b + l_b)
            frac = (2 * 157.0 * s_b - 140.0 * x_b) / (2 * 157.0 * s_b + 140.0 * s_b)
            frac = min(max(frac, 0.0), 1.0)
            if bal_frac is not None:
                frac = float(bal_frac)

            def split_dma(eng, out_ap, in_ap, F):
                # first frac of columns on `eng`, rest on SWDGE (gpsimd)
                cut = max(0, int(F * frac)) & ~63
                if cut > 0:
                    eng.dma_start(out_ap[:, 0:cut], in_ap[:, 0:cut])
                if cut < F:
                    nc.gpsimd.dma_start(out_ap[:, cut:F], in_ap[:, cut:F])

            tile_idx = [0]

            def do_tile(c0, F):
                ti = tile_idx[0]
                tile_idx[0] += 1
                ld_n_t = ld_n
                st_l_t = st_l
                if ring_mode == "balt":
                    # 1-in-8 tiles keep n/lik on the HW ring, rest on SWDGE
                    ld_n_t = nc.sync if ti % 8 == 0 else nc.gpsimd
                    st_l_t = nc.scalar if ti % 8 == 4 else nc.gpsimd
                xt = iop.tile([128, F], f16, tag="xt")
                ld_x.dma_start(xt[:], x_d[:, c0:c0 + F])
                nt = iop.tile([128, F], ndt, tag="nt")
                if bal3:
                    split_dma(ld_n, nt, n_d[:, c0:c0 + F], F)
                else:
                    ld_n_t.dma_start(nt[:], n_d[:, c0:c0 + F])

                if stage == 0:
                    st_o.dma_start(o_d[:, c0:c0 + F], xt[:])
                    st_l.dma_start(lik_d[:, c0:c0 + F], nt[:])
                    return

                ot = iop.tile([128, F], f16, tag="ot")
                nc.vector.tensor_tensor(ot[:], xt[:], nt[:], ALU.add)
                st_o.dma_start(o_d[:, c0:c0 + F], ot[:])

                likt = iop.tile([128, F], ldt, tag="likt")
                if variant == "sig2":
                    s1 = workp.tile([128, F], f32, tag="s1")
                    nc.scalar.activation(s1[:], ot[:], AF.Sigmoid,
                                         bias=bu_ap, scale=a_ap)
                    s2 = workp.tile([128, F], f32, tag="s2")
                    nc.scalar.activation(s2[:], ot[:], AF.Sigmoid,
                                         bias=bl_ap, scale=a_ap)
                    if lik_dtype in ("i8", "u8"):
                        df = workp.tile([128, F], f32, tag="df")
                        sub.tensor_tensor(df[:], s1[:], s2[:], ALU.subtract)
                        nc.vector.tensor_scalar(likt[:], df[:],
                                                LIK_SCALES[lik_dtype],
                                                None, ALU.mult)
                    else:
                        sub.tensor_tensor(likt[:], s1[:], s2[:], ALU.subtract)
                else:  # "tanh": lik = 2h*sigma'(t) = (h/2)*(1 - tanh(t/2)^2)
                    s1 = workp.tile([128, F], f32, tag="s1")
                    nc.scalar.activation(s1[:], ot[:], AF.Tanh,
                                         bias=d2_ap, scale=a2_ap)
                    s2 = workp.tile([128, F], f32, tag="s2")
                    nc.gpsimd.tensor_tensor(s2[:], s1[:], s1[:], ALU.mult)
                    nc.vector.tensor_scalar(likt[:], s2[:], nh2_ap, h2_ap,
                                            ALU.mult, ALU.add)
                if bal3:
                    split_dma(st_l, lik_d[:, c0:c0 + F], likt, F)
                else:
                    st_l_t.dma_start(lik_d[:, c0:c0 + F], likt[:])

            for _ in range(reps):
                c0 = 0
                for _ in range(n_full):
                    do_tile(c0, tile_f)
                    c0 += tile_f
                if tail_f:
                    do_tile(c0, tail_f)

    nc.compile()
    return nc


def _get_nc():
    if "nc" not in _CACHE:
        _CACHE["nc"] = _build_bass()
    return _CACHE["nc"]


def _make_prm(inputs):
    a64, d64 = _collapse_affine(inputs)
    h64 = 0.5 * a64
    prm = np.zeros((128, 8), dtype=np.float32)
    idx = np.arange(128) % C
    prm[:, 0] = a64[idx]
    prm[:, 1] = (d64 + h64)[idx]
    prm[:, 2] = (d64 - h64)[idx]
    prm[:, 3] = (0.5 * a64)[idx]
    prm[:, 4] = (0.5 * d64)[idx]
    prm[:, 5] = (0.5 * h64)[idx]
    prm[:, 6] = (-0.5 * h64)[idx]
    return prm


def _make_in_maps(inputs, n_dtype=None, cols_pad=None, **_ignored):
    """Shard + pack: per-core [62500,64] -> transposed [128, 31250] fp16."""
    if n_dtype is None:
        n_dtype = N_DTYPE
    if cols_pad is None:
        cols_pad = COLS_PAD
    if n_dtype == "f8":
        import ml_dtypes
        ndt = ml_dtypes.float8_e4m3fn
    else:
        ndt = np.float16
    x = np.asarray(inputs["inputs"], dtype=np.float32)
    nz = np.asarray(inputs["noise"], dtype=np.float32)
    x2 = x.reshape(N_CORES, COLS, 128).transpose(0, 2, 1).astype(np.float16)
    n2 = nz.reshape(N_CORES, COLS, 128).transpose(0, 2, 1).astype(ndt)
    if cols_pad != COLS:
        xp = np.zeros((N_CORES, 128, cols_pad), dtype=np.float16)
        xp[:, :, :COLS] = x2
        np_ = np.zeros((N_CORES, 128, cols_pad), dtype=ndt)
        np_[:, :, :COLS] = n2
        x2, n2 = xp, np_
    prm = _make_prm(inputs)
    return [{"x": x2[i], "n": n2[i], "prm": prm} for i in range(N_CORES)]


def _reference_numpy(inputs):
    """Faithful float32 numpy fallback for the general (f != 0) case."""
    x = np.asarray(inputs["inputs"], dtype=np.float32)
    nz = np.asarray(inputs["noise"], dtype=np.float32)
    o = x + nz
    xt = o.T[:, None, :]  # (C, 1, N)

    def softplus32(v):
        v = v.astype(np.float32)
        return (np.log1p(np.exp(-np.abs(v))) + np.maximum(v, 0)).astype(np.float32)

    def logits_cum(z):
        logits = z.astype(np.float32)
        for i in range(4):
            W = softplus32(np.asarray(inputs[f"m{i}"]))
            b = np.asarray(inputs[f"b{i}"], dtype=np.float32)
            f = np.asarray(inputs[f"f{i}"], dtype=np.float32)
            logits = np.einsum("cij,cjn->cin", W, logits).astype(np.float32) + b
            logits = logits + np.tanh(f) * np.tanh(logits)
        return logits.astype(np.float32)

    lower = logits_cum(xt - np.float32(0.5))
    upper = logits_cum(xt + np.float32(0.5))
    sign = -np.sign(lower + upper)

    def sig(v):
        return (1.0 / (1.0 + np.exp(-v.astype(np.float64)))).astype(np.float32)

    lik = np.abs(sig(sign * upper) - sig(sign * lower))
    lik = lik.reshape(C, -1).T
    lik = np.maximum(lik, np.float32(1e-9))
    return o, lik


def kernel(**inputs):
    x = np.asarray(inputs["inputs"], dtype=np.float32)

    f_zero = all(np.all(np.asarray(inputs[f"f{i}"]) == 0) for i in range(4))
    if x.shape != (N_TOTAL, C) or not f_zero:
        return _reference_numpy(inputs)

    in_maps = _make_in_maps(inputs)
    res = None
    for attempt in range(2):
        try:
            from concourse.bass_utils import run_bass_kernel_spmd
            nc = _get_nc()
            res = run_bass_kernel_spmd(nc, in_maps,
                                       core_ids=list(range(N_CORES)))
            break
        except Exception:
            _CACHE.pop("nc", None)  # rebuild on retry
            if attempt == 1:
                # device unusable -- return the faithful host computation
                return _reference_numpy(inputs)
    _CACHE["last_results"] = res

    o2 = np.stack([np.asarray(r["o"])[:, :COLS] for r in res.results])  # [8,128,COLS]
    l2 = np.stack([np.asarray(r["lik"])[:, :COLS] for r in res.results])
    o = o2.transpose(0, 2, 1).reshape(N_TOTAL, C).astype(np.float32)
    lik = l2.transpose(0, 2, 1).reshape(N_TOTAL, C).astype(np.float32)
    if LIK_DTYPE in ("i8", "u8"):
        lik *= np.float32(1.0 / LIK_SCALES[LIK_DTYPE])
    return o, lik


# revision 31
# speedup vs baseline: 1.0400x; 1.0400x over previous
"""EntropyBottleneck forward kernel for Trainium2 (8 NeuronCores, data-parallel).

Math: with the per-channel gate params f == 0 (always true for this problem's
inputs), each _logits_cumulative layer is affine, so the whole 4-layer chain
collapses to t = a_c * x + d_c per channel c (a_c ~= 0.125). The likelihood is

    lik = | sigmoid(s*(t+h)) - sigmoid(s*(t-h)) |,  s = -sign(2t), h = a_c/2
        =   sigmoid(t+h) - sigmoid(t-h)            (identical in exact math)

The sign trick in the reference only guards against fp32 cancellation when
both sigmoids saturate; here |t| <= ~4.5 (a ~= 0.125, |o| <= ~25, |d| <= ~1.5)
so sigma ranges over ~[0.01, 0.99] and the direct fp32 difference is accurate.
lik in [2.7e-3, h/2], so the 1e-9 low_bound clip never binds and lik fits a
uint8 fixed-point format exactly.

Layout: the host packs each core's [62500, 64] slab as a transposed
[128, 31250] array, which puts the channel index on the SBUF partition axis
for free (partition p holds channel p % 64). All per-channel params are then
plain per-partition [128,1] scale/bias vectors: no TensorE transposes and no
PSUM use at all. Device work per element: o = x + n (DVE), two sigmoids (ACT
with per-partition scale a and biases d+-h, fp32 internally), subtract +
uint8 quantize (DVE). All compute hides behind the DMAs.

The kernel is DMA-bound, so I/O dtypes are chosen as small as the 2e-2
norm-relative-error gate safely allows (measured ~1.7e-3): x fp16, n fp8
(e4m3), o fp16, lik uint8 (scale 8064 ~= 2*127/h; values <= 252). That is
24 MB/core per invocation vs 64 MB for fp32. DMA uses both HWDGE rings (SP:
x-load + a sliver of n; ACT: o-store + a sliver of lik) plus SWDGE (gpsimd)
for ~92% of the 1-byte n/lik streams, byte-balanced across the three queues
("bal3"); 4096-column tiles give 8 KB (4 KB) per-partition segments, the
measured DMA sweet spot. Aggregate achieved ~320 GB/s/core; pure-DMA
ablations put the machine ceiling at ~330-340 GB/s/core.

Sharding: data-parallel over points N across the 8 cores; tiny params
replicated; no cross-core communication.
"""

import numpy as np

N_TOTAL = 500000
C = 64
N_CORES = 8
ROWS_PER_CORE = N_TOTAL // N_CORES          # 62500
ELEMS = ROWS_PER_CORE * C                   # 4,000,000 per core
CHUNKS = ELEMS // 128                       # 31250 = columns of the [128, COLS] view
COLS = CHUNKS
TILE_F = 4096                               # main tile width (8 KB/partition fp16)
N_FULL_TILES = COLS // TILE_F               # 15
TAIL_F = COLS - N_FULL_TILES * TILE_F       # 530

VARIANT = "sig2"        # "sig2" = exact two-sigmoid; "tanh" = 2h*sigma'(t)
RING = "bal3"
N_DTYPE = "f8"          # "f16" | "f8" (float8 e4m3 noise)
LIK_DTYPE = "u8"        # "f16" | "i8" | "u8" (int likelihood, scale 1/LIK_SCALE)
LIK_SCALES = {"i8": 4032.0, "u8": 8064.0}   # lik <= h/2 ~= 0.03125
COLS_PAD = COLS         # dram row pitch (power-of-2 padding measured WORSE)
NL_SPAN = 2             # n-load/lik-store span 2 tiles -> 8KB 1-byte segments

_CACHE: dict = {}


def _softplus64(x):
    return np.log1p(np.exp(-np.abs(x))) + np.maximum(x, 0.0)


def _collapse_affine(inputs):
    """Fold the 4 affine layers into per-channel (a, d) in float64."""
    alpha = None
    beta = None
    for i in range(4):
        W = _softplus64(np.asarray(inputs[f"m{i}"], dtype=np.float64))  # (C, fo, fi)
        bb = np.asarray(inputs[f"b{i}"], dtype=np.float64)[:, :, 0]     # (C, fo)
        if i == 0:
            alpha = W[:, :, 0]
            beta = bb
        else:
            alpha = np.einsum("cij,cj->ci", W, alpha)
            beta = np.einsum("cij,cj->ci", W, beta) + bb
    return alpha[:, 0], beta[:, 0]  # (C,), (C,)


def _build_bass(reps=1, variant=None, ring_mode=None, stage=4, sub_eng="vector",
                tile_f=None, n_dtype=None, lik_dtype=None, cols_pad=None,
                bal_frac=None, io_bufs=4, sp_pkt=0, nl_span=None, **_legacy):
    if nl_span is None:
        nl_span = NL_SPAN
    # stage: 0 = pure DMA passthrough (o <- x, lik <- n), 4 = full kernel
    if variant is None:
        variant = VARIANT
    if ring_mode is None:
        ring_mode = RING
    if tile_f is None:
        tile_f = TILE_F
    if n_dtype is None:
        n_dtype = N_DTYPE
    if lik_dtype is None:
        lik_dtype = LIK_DTYPE
    if cols_pad is None:
        cols_pad = COLS_PAD
    n_full, tail_f = divmod(COLS, tile_f)
    import concourse.bacc as bacc
    import concourse.mybir as mybir
    from concourse.mybir import ActivationFunctionType as AF
    from concourse.mybir import AluOpType as ALU
    from concourse.tile import TileContext

    f16 = mybir.dt.float16
    f32 = mybir.dt.float32
    nc = bacc.Bacc("TRN2", target_bir_lowering=False, debug=False,
                   enable_asserts=False, num_devices=N_CORES)

    f8 = mybir.dt.float8e4
    ndt = f8 if n_dtype == "f8" else f16
    ldt = {"i8": mybir.dt.int8, "u8": mybir.dt.uint8, "f16": f16}[lik_dtype]
    assert stage != 0 or (n_dtype == "f16" and lik_dtype == "f16")

    CP = cols_pad
    x_d = nc.dram_tensor("x", [128, CP], f16, kind="ExternalInput")
    n_d = nc.dram_tensor("n", [128, CP], ndt, kind="ExternalInput")
    prm_d = nc.dram_tensor("prm", [128, 8], f32, kind="ExternalInput")
    o_d = nc.dram_tensor("o", [128, CP], f16, kind="ExternalOutput")
    lik_d = nc.dram_tensor("lik", [128, CP], ldt, kind="ExternalOutput")

    with TileContext(nc) as tc:
        with (
            tc.tile_pool(name="const", bufs=1) as constp,
            tc.tile_pool(name="io", bufs=io_bufs) as iop,
            tc.tile_pool(name="nl", bufs=(2 if nl_span > 1 else io_bufs)) as nlp,
            tc.tile_pool(name="work", bufs=2) as workp,
        ):
            prm = constp.tile([128, 8], f32)
            nc.sync.dma_start(prm[:], prm_d[:, :])
            a_ap = prm[:, 0:1]       # a
            bu_ap = prm[:, 1:2]      # d + h
            bl_ap = prm[:, 2:3]      # d - h
            a2_ap = prm[:, 3:4]      # a / 2
            d2_ap = prm[:, 4:5]      # d / 2
            h2_ap = prm[:, 5:6]      # h / 2
            nh2_ap = prm[:, 6:7]     # -h / 2

            # engine per DMA stream: (x-load, n-load, o-store, lik-store);
            # "bal3" splits n-load/lik-store between a HW ring and SWDGE so
            # bytes balance across SP/ACT/SWDGE in proportion to their rates.
            bal3 = ring_mode in ("bal3", "bal3s", "bal3m")
            if ring_mode == "sw7":
                engs = (nc.sync, nc.gpsimd, nc.scalar, nc.gpsimd)
            elif ring_mode == "sw2":
                engs = (nc.sync, nc.gpsimd, nc.scalar, nc.sync)
            elif ring_mode in ("ls", "bal3", "balt", "bal3m"):
                engs = (nc.sync, nc.sync, nc.scalar, nc.scalar)
            elif ring_mode == "bal3s":
                engs = (nc.scalar, nc.scalar, nc.sync, nc.sync)
            elif ring_mode == "sw0":
                engs = (nc.sync, nc.gpsimd, nc.scalar, nc.gpsimd)
            elif ring_mode == "swx":
                engs = (nc.gpsimd, nc.sync, nc.scalar, nc.gpsimd)
            else:
                raise ValueError(ring_mode)
            ld_x, ld_n, st_o, st_l = engs
            # HW-ring engine for the n/lik slivers in bal3 modes; "bal3m"
            # crosses them (n sliver on the store ring, lik on the load ring)
            sliver_n, sliver_l = ld_n, st_l
            if ring_mode == "bal3m":
                sliver_n, sliver_l = st_o, ld_x

            sub = {"vector": nc.vector, "gpsimd": nc.gpsimd}[sub_eng]

            # bal3 split: fraction `frac` of the n-load (and lik-store) rides
            # the HW ring with x (o), the rest goes to SWDGE, chosen so that
            # ring byte-time balances at rates SP=ACT=157, SWDGE=140 GB/s:
            #   (x_b + frac*n_b)/157 = (1-frac)*(n_b + l_b)/140
            x_b = 2.0
            n_b = 1.0 if n_dtype == "f8" else 2.0
            l_b = 1.0 if lik_dtype in ("i8", "u8") else 2.0
            s_b = 0.5 * (n_b + l_b)
            frac = (2 * 157.0 * s_b - 140.0 * x_b) / (2 * 157.0 * s_b + 140.0 * s_b)
            frac = min(max(frac, 0.0), 1.0)
            if bal_frac is not None:
                frac = float(bal_frac)

            def split_dma(eng, out_ap, in_ap, F):
                # first frac of columns on `eng`, rest on SWDGE (gpsimd)
                cut = max(0, int(F * frac)) & ~63
                if cut > 0:
                    eng.dma_start(out_ap[:, 0:cut], in_ap[:, 0:cut],
                                  single_packet=bool(sp_pkt))
                if cut < F:
                    nc.gpsimd.dma_start(out_ap[:, cut:F], in_ap[:, cut:F],
                                        single_packet=bool(sp_pkt))

            tile_idx = [0]
            span_state = {}

            def do_tile(c0, F, F_next=0):
                # nl_span=2: n-load and lik-store cover this tile plus the
                # next one (wider per-partition DMA segments for 1B dtypes)
                ti = tile_idx[0]
                tile_idx[0] += 1
                ld_n_t = ld_n
                st_l_t = st_l
                if ring_mode == "balt":
                    # 1-in-8 tiles keep n/lik on the HW ring, rest on SWDGE
                    ld_n_t = nc.sync if ti % 8 == 0 else nc.gpsimd
                    st_l_t = nc.scalar if ti % 8 == 4 else nc.gpsimd
                xt = iop.tile([128, F], f16, tag="xt")
                ld_x.dma_start(xt[:], x_d[:, c0:c0 + F],
                               single_packet=bool(sp_pkt))
                pair_lead = nl_span > 1 and ti % 2 == 0 and F_next > 0
                pair_trail = nl_span > 1 and ti % 2 == 1
                if pair_trail:
                    nt_full, n_off = span_state["nt"], span_state["F0"]
                    nt = nt_full[:, n_off:n_off + F]
                else:
                    W = F + F_next if pair_lead else F
                    nt_full = nlp.tile([128, W], ndt, tag="nt")
                    if bal3:
                        split_dma(sliver_n, nt_full, n_d[:, c0:c0 + W], W)
                    else:
                        ld_n_t.dma_start(nt_full[:], n_d[:, c0:c0 + W])
                    nt = nt_full[:, 0:F]
                    if pair_lead:
                        span_state["nt"] = nt_full
                        span_state["F0"] = F

                if stage == 0:
                    st_o.dma_start(o_d[:, c0:c0 + F], xt[:])
                    st_l.dma_start(lik_d[:, c0:c0 + F], nt[:])
                    return

                ot = iop.tile([128, F], f16, tag="ot")
                nc.vector.tensor_tensor(ot[:], xt[:], nt[:], ALU.add)
                st_o.dma_start(o_d[:, c0:c0 + F], ot[:],
                               single_packet=bool(sp_pkt))

                if pair_trail:
                    likt_full, l_off = span_state["likt"], span_state["F0"]
                    likt = likt_full[:, l_off:l_off + F]
                elif pair_lead:
                    likt_full = nlp.tile([128, F + F_next], ldt, tag="likt")
                    span_state["likt"] = likt_full
                    likt = likt_full[:, 0:F]
                else:
                    likt_full = nlp.tile([128, F], ldt, tag="likt")
                    likt = likt_full
                if variant == "sig2":
                    s1 = workp.tile([128, F], f32, tag="s1")
                    nc.scalar.activation(s1[:], ot[:], AF.Sigmoid,
                                         bias=bu_ap, scale=a_ap)
                    s2 = workp.tile([128, F], f32, tag="s2")
                    nc.scalar.activation(s2[:], ot[:], AF.Sigmoid,
                                         bias=bl_ap, scale=a_ap)
                    if lik_dtype in ("i8", "u8"):
                        df = workp.tile([128, F], f32, tag="df")
                        sub.tensor_tensor(df[:], s1[:], s2[:], ALU.subtract)
                        nc.vector.tensor_scalar(likt[:], df[:],
                                                LIK_SCALES[lik_dtype],
                                                None, ALU.mult)
                    else:
                        sub.tensor_tensor(likt[:], s1[:], s2[:], ALU.subtract)
                else:  # "tanh": lik = 2h*sigma'(t) = (h/2)*(1 - tanh(t/2)^2)
                    s1 = workp.tile([128, F], f32, tag="s1")
                    nc.scalar.activation(s1[:], ot[:], AF.Tanh,
                                         bias=d2_ap, scale=a2_ap)
                    s2 = workp.tile([128, F], f32, tag="s2")
                    nc.gpsimd.tensor_tensor(s2[:], s1[:], s1[:], ALU.mult)
                    nc.vector.tensor_scalar(likt[:], s2[:], nh2_ap, h2_ap,
                                            ALU.mult, ALU.add)
                if pair_lead:
                    pass  # store issued by the trailing tile of the pair
                elif pair_trail:
                    W = l_off + F
                    c0p = c0 - l_off
                    if bal3:
                        split_dma(sliver_l, lik_d[:, c0p:c0p + W],
                                  likt_full, W)
                    else:
                        st_l_t.dma_start(lik_d[:, c0p:c0p + W], likt_full[:])
                elif bal3:
                    split_dma(sliver_l, lik_d[:, c0:c0 + F], likt, F)
                else:
                    st_l_t.dma_start(lik_d[:, c0:c0 + F], likt[:])

            widths = [tile_f] * n_full + ([tail_f] if tail_f else [])
            for _ in range(reps):
                c0 = 0
                for j, F in enumerate(widths):
                    F_next = widths[j + 1] if j + 1 < len(widths) else 0
                    do_tile(c0, F, F_next)
                    c0 += F
                tile_idx[0] += len(widths) % 2  # keep pair phase aligned

    nc.compile()
    return nc


def _get_nc():
    if "nc" not in _CACHE:
        _CACHE["nc"] = _build_bass()
    return _CACHE["nc"]


def _make_prm(inputs):
    a64, d64 = _collapse_affine(inputs)
    h64 = 0.5 * a64
    prm = np.zeros((128, 8), dtype=np.float32)
    idx = np.arange(128) % C
    prm[:, 0] = a64[idx]
    prm[:, 1] = (d64 + h64)[idx]
    prm[:, 2] = (d64 - h64)[idx]
    prm[:, 3] = (0.5 * a64)[idx]
    prm[:, 4] = (0.5 * d64)[idx]
    prm[:, 5] = (0.5 * h64)[idx]
    prm[:, 6] = (-0.5 * h64)[idx]
    return prm


def _make_in_maps(inputs, n_dtype=None, cols_pad=None, **_ignored):
    """Shard + pack: per-core [62500,64] -> transposed [128, 31250] fp16."""
    if n_dtype is None:
        n_dtype = N_DTYPE
    if cols_pad is None:
        cols_pad = COLS_PAD
    if n_dtype == "f8":
        import ml_dtypes
        ndt = ml_dtypes.float8_e4m3fn
    else:
        ndt = np.float16
    x = np.asarray(inputs["inputs"], dtype=np.float32)
    nz = np.asarray(inputs["noise"], dtype=np.float32)
    x2 = x.reshape(N_CORES, COLS, 128).transpose(0, 2, 1).astype(np.float16)
    n2 = nz.reshape(N_CORES, COLS, 128).transpose(0, 2, 1).astype(ndt)
    if cols_pad != COLS:
        xp = np.zeros((N_CORES, 128, cols_pad), dtype=np.float16)
        xp[:, :, :COLS] = x2
        np_ = np.zeros((N_CORES, 128, cols_pad), dtype=ndt)
        np_[:, :, :COLS] = n2
        x2, n2 = xp, np_
    prm = _make_prm(inputs)
    return [{"x": x2[i], "n": n2[i], "prm": prm} for i in range(N_CORES)]


def _reference_numpy(inputs):
    """Faithful float32 numpy fallback for the general (f != 0) case."""
    x = np.asarray(inputs["inputs"], dtype=np.float32)
    nz = np.asarray(inputs["noise"], dtype=np.float32)
    o = x + nz
    xt = o.T[:, None, :]  # (C, 1, N)

    def softplus32(v):
        v = v.astype(np.float32)
        return (np.log1p(np.exp(-np.abs(v))) + np.maximum(v, 0)).astype(np.float32)

    def logits_cum(z):
        logits = z.astype(np.float32)
        for i in range(4):
            W = softplus32(np.asarray(inputs[f"m{i}"]))
            b = np.asarray(inputs[f"b{i}"], dtype=np.float32)
            f = np.asarray(inputs[f"f{i}"], dtype=np.float32)
            logits = np.einsum("cij,cjn->cin", W, logits).astype(np.float32) + b
            logits = logits + np.tanh(f) * np.tanh(logits)
        return logits.astype(np.float32)

    lower = logits_cum(xt - np.float32(0.5))
    upper = logits_cum(xt + np.float32(0.5))
    sign = -np.sign(lower + upper)

    def sig(v):
        return (1.0 / (1.0 + np.exp(-v.astype(np.float64)))).astype(np.float32)

    lik = np.abs(sig(sign * upper) - sig(sign * lower))
    lik = lik.reshape(C, -1).T
    lik = np.maximum(lik, np.float32(1e-9))
    return o, lik


def kernel(**inputs):
    x = np.asarray(inputs["inputs"], dtype=np.float32)

    f_zero = all(np.all(np.asarray(inputs[f"f{i}"]) == 0) for i in range(4))
    if x.shape != (N_TOTAL, C) or not f_zero:
        return _reference_numpy(inputs)

    in_maps = _make_in_maps(inputs)
    res = None
    for attempt in range(2):
        try:
            from concourse.bass_utils import run_bass_kernel_spmd
            nc = _get_nc()
            res = run_bass_kernel_spmd(nc, in_maps,
                                       core_ids=list(range(N_CORES)))
            break
        except Exception:
            _CACHE.pop("nc", None)  # rebuild on retry
            if attempt == 1:
                # device unusable -- return the faithful host computation
                return _reference_numpy(inputs)
    _CACHE["last_results"] = res

    o2 = np.stack([np.asarray(r["o"])[:, :COLS] for r in res.results])  # [8,128,COLS]
    l2 = np.stack([np.asarray(r["lik"])[:, :COLS] for r in res.results])
    o = o2.transpose(0, 2, 1).reshape(N_TOTAL, C).astype(np.float32)
    lik = l2.transpose(0, 2, 1).reshape(N_TOTAL, C).astype(np.float32)
    if LIK_DTYPE in ("i8", "u8"):
        lik *= np.float32(1.0 / LIK_SCALES[LIK_DTYPE])
    return o, lik


# revision 32
# speedup vs baseline: 1.0567x; 1.0161x over previous
"""EntropyBottleneck forward kernel for Trainium2 (8 NeuronCores, data-parallel).

Math: with the per-channel gate params f == 0 (always true for this problem's
inputs), each _logits_cumulative layer is affine, so the whole 4-layer chain
collapses to t = a_c * x + d_c per channel c (a_c ~= 0.125). The likelihood is

    lik = | sigmoid(s*(t+h)) - sigmoid(s*(t-h)) |,  s = -sign(2t), h = a_c/2
        =   sigmoid(t+h) - sigmoid(t-h)            (identical in exact math)

The sign trick in the reference only guards against fp32 cancellation when
both sigmoids saturate; here |t| <= ~4.5 (a ~= 0.125, |o| <= ~25, |d| <= ~1.5)
so sigma ranges over ~[0.01, 0.99] and the direct fp32 difference is accurate.
lik in [2.7e-3, h/2], so the 1e-9 low_bound clip never binds and lik fits a
uint8 fixed-point format exactly.

Layout: the host packs each core's [62500, 64] slab as a transposed
[128, 31250] array, which puts the channel index on the SBUF partition axis
for free (partition p holds channel p % 64). All per-channel params are then
plain per-partition [128,1] scale/bias vectors: no TensorE transposes and no
PSUM use at all. Device work per element: o = x + n (DVE), two sigmoids (ACT
with per-partition scale a and biases d+-h, fp32 internally), subtract +
uint8 quantize (DVE). All compute hides behind the DMAs.

The kernel is DMA-bound, so I/O dtypes are chosen as small as the 2e-2
norm-relative-error gate safely allows (measured ~1.7e-3): x fp16, n fp8
(e4m3), o fp16, lik uint8 (scale 8064 ~= 2*127/h; values <= 252). That is
24 MB/core per invocation vs 64 MB for fp32. DMA uses both HWDGE rings (SP:
x-load + a sliver of n; ACT: o-store + a sliver of lik) plus SWDGE (gpsimd)
for ~92% of the 1-byte n/lik streams, byte-balanced across the three queues
("bal3"). Compute runs on 4096-column tiles (8 KB fp16 per-partition
segments, the measured DMA sweet spot); the 1-byte n-loads/lik-stores span
two tiles each ("nl_span=2") so their segments are also 8 KB. Power-of-2 row
pitch measured ~9% slower (HBM channel aliasing), so the natural 62500 B
stride is kept. Aggregate achieved ~320 GB/s/core; pure-DMA ablations put
the machine ceiling at ~330-340 GB/s/core.

Sharding: data-parallel over points N across the 8 cores; tiny params
replicated; no cross-core communication.
"""

import numpy as np

N_TOTAL = 500000
C = 64
N_CORES = 8
ROWS_PER_CORE = N_TOTAL // N_CORES          # 62500
ELEMS = ROWS_PER_CORE * C                   # 4,000,000 per core
CHUNKS = ELEMS // 128                       # 31250 = columns of the [128, COLS] view
COLS = CHUNKS
TILE_F = 4096                               # main tile width (8 KB/partition fp16)
N_FULL_TILES = COLS // TILE_F               # 15
TAIL_F = COLS - N_FULL_TILES * TILE_F       # 530

VARIANT = "sig2"        # "sig2" = exact two-sigmoid; "tanh" = 2h*sigma'(t)
RING = "bal3"
N_DTYPE = "f8"          # "f16" | "f8" (float8 e4m3 noise)
LIK_DTYPE = "u8"        # "f16" | "i8" | "u8" (int likelihood, scale 1/LIK_SCALE)
LIK_SCALES = {"i8": 4032.0, "u8": 8064.0}   # lik <= h/2 ~= 0.03125
COLS_PAD = COLS         # dram row pitch (power-of-2 padding measured WORSE)
NL_SPAN = 2             # n-load/lik-store span 2 tiles -> 8KB 1-byte segments

_CACHE: dict = {}


def _softplus64(x):
    return np.log1p(np.exp(-np.abs(x))) + np.maximum(x, 0.0)


def _collapse_affine(inputs):
    """Fold the 4 affine layers into per-channel (a, d) in float64."""
    alpha = None
    beta = None
    for i in range(4):
        W = _softplus64(np.asarray(inputs[f"m{i}"], dtype=np.float64))  # (C, fo, fi)
        bb = np.asarray(inputs[f"b{i}"], dtype=np.float64)[:, :, 0]     # (C, fo)
        if i == 0:
            alpha = W[:, :, 0]
            beta = bb
        else:
            alpha = np.einsum("cij,cj->ci", W, alpha)
            beta = np.einsum("cij,cj->ci", W, beta) + bb
    return alpha[:, 0], beta[:, 0]  # (C,), (C,)


def _build_bass(reps=1, variant=None, ring_mode=None, stage=4, sub_eng="vector",
                tile_f=None, n_dtype=None, lik_dtype=None, cols_pad=None,
                bal_frac=None, io_bufs=4, sp_pkt=0, nl_span=None, **_legacy):
    if nl_span is None:
        nl_span = NL_SPAN
    # stage: 0 = pure DMA passthrough (o <- x, lik <- n), 4 = full kernel
    if variant is None:
        variant = VARIANT
    if ring_mode is None:
        ring_mode = RING
    if tile_f is None:
        tile_f = TILE_F
    if n_dtype is None:
        n_dtype = N_DTYPE
    if lik_dtype is None:
        lik_dtype = LIK_DTYPE
    if cols_pad is None:
        cols_pad = COLS_PAD
    n_full, tail_f = divmod(COLS, tile_f)
    import concourse.bacc as bacc
    import concourse.mybir as mybir
    from concourse.mybir import ActivationFunctionType as AF
    from concourse.mybir import AluOpType as ALU
    from concourse.tile import TileContext

    f16 = mybir.dt.float16
    f32 = mybir.dt.float32
    nc = bacc.Bacc("TRN2", target_bir_lowering=False, debug=False,
                   enable_asserts=False, num_devices=N_CORES)

    f8 = mybir.dt.float8e4
    ndt = f8 if n_dtype == "f8" else f16
    ldt = {"i8": mybir.dt.int8, "u8": mybir.dt.uint8, "f16": f16}[lik_dtype]
    assert stage != 0 or (n_dtype == "f16" and lik_dtype == "f16")

    CP = cols_pad
    x_d = nc.dram_tensor("x", [128, CP], f16, kind="ExternalInput")
    n_d = nc.dram_tensor("n", [128, CP], ndt, kind="ExternalInput")
    prm_d = nc.dram_tensor("prm", [128, 8], f32, kind="ExternalInput")
    o_d = nc.dram_tensor("o", [128, CP], f16, kind="ExternalOutput")
    lik_d = nc.dram_tensor("lik", [128, CP], ldt, kind="ExternalOutput")

    with TileContext(nc) as tc:
        with (
            tc.tile_pool(name="const", bufs=1) as constp,
            tc.tile_pool(name="io", bufs=io_bufs) as iop,
            tc.tile_pool(name="nl", bufs=(2 if nl_span > 1 else io_bufs)) as nlp,
            tc.tile_pool(name="work", bufs=2) as workp,
        ):
            prm = constp.tile([128, 8], f32)
            nc.sync.dma_start(prm[:], prm_d[:, :])
            a_ap = prm[:, 0:1]       # a
            bu_ap = prm[:, 1:2]      # d + h
            bl_ap = prm[:, 2:3]      # d - h
            a2_ap = prm[:, 3:4]      # a / 2
            d2_ap = prm[:, 4:5]      # d / 2
            h2_ap = prm[:, 5:6]      # h / 2
            nh2_ap = prm[:, 6:7]     # -h / 2

            # engine per DMA stream: (x-load, n-load, o-store, lik-store);
            # "bal3" splits n-load/lik-store between a HW ring and SWDGE so
            # bytes balance across SP/ACT/SWDGE in proportion to their rates.
            bal3 = ring_mode in ("bal3", "bal3s", "bal3m")
            if ring_mode == "sw7":
                engs = (nc.sync, nc.gpsimd, nc.scalar, nc.gpsimd)
            elif ring_mode == "sw2":
                engs = (nc.sync, nc.gpsimd, nc.scalar, nc.sync)
            elif ring_mode in ("ls", "bal3", "balt", "bal3m"):
                engs = (nc.sync, nc.sync, nc.scalar, nc.scalar)
            elif ring_mode == "bal3s":
                engs = (nc.scalar, nc.scalar, nc.sync, nc.sync)
            elif ring_mode == "sw0":
                engs = (nc.sync, nc.gpsimd, nc.scalar, nc.gpsimd)
            elif ring_mode == "swx":
                engs = (nc.gpsimd, nc.sync, nc.scalar, nc.gpsimd)
            else:
                raise ValueError(ring_mode)
            ld_x, ld_n, st_o, st_l = engs
            # HW-ring engine for the n/lik slivers in bal3 modes; "bal3m"
            # crosses them (n sliver on the store ring, lik on the load ring)
            sliver_n, sliver_l = ld_n, st_l
            if ring_mode == "bal3m":
                sliver_n, sliver_l = st_o, ld_x

            sub = {"vector": nc.vector, "gpsimd": nc.gpsimd}[sub_eng]

            # bal3 split: fraction `frac` of the n-load (and lik-store) rides
            # the HW ring with x (o), the rest goes to SWDGE, chosen so that
            # ring byte-time balances at rates SP=ACT=157, SWDGE=140 GB/s:
            #   (x_b + frac*n_b)/157 = (1-frac)*(n_b + l_b)/140
            x_b = 2.0
            n_b = 1.0 if n_dtype == "f8" else 2.0
            l_b = 1.0 if lik_dtype in ("i8", "u8") else 2.0
            s_b = 0.5 * (n_b + l_b)
            frac = (2 * 157.0 * s_b - 140.0 * x_b) / (2 * 157.0 * s_b + 140.0 * s_b)
            frac = min(max(frac, 0.0), 1.0)
            if bal_frac is not None:
                frac = float(bal_frac)

            def split_dma(eng, out_ap, in_ap, F):
                # first frac of columns on `eng`, rest on SWDGE (gpsimd)
                cut = max(0, int(F * frac)) & ~63
                if cut > 0:
                    eng.dma_start(out_ap[:, 0:cut], in_ap[:, 0:cut],
                                  single_packet=bool(sp_pkt))
                if cut < F:
                    nc.gpsimd.dma_start(out_ap[:, cut:F], in_ap[:, cut:F],
                                        single_packet=bool(sp_pkt))

            tile_idx = [0]
            span_state = {}

            def do_tile(c0, F, F_next=0):
                # nl_span=2: n-load and lik-store cover this tile plus the
                # next one (wider per-partition DMA segments for 1B dtypes)
                ti = tile_idx[0]
                tile_idx[0] += 1
                ld_n_t = ld_n
                st_l_t = st_l
                if ring_mode == "balt":
                    # 1-in-8 tiles keep n/lik on the HW ring, rest on SWDGE
                    ld_n_t = nc.sync if ti % 8 == 0 else nc.gpsimd
                    st_l_t = nc.scalar if ti % 8 == 4 else nc.gpsimd
                xt = iop.tile([128, F], f16, tag="xt")
                ld_x.dma_start(xt[:], x_d[:, c0:c0 + F],
                               single_packet=bool(sp_pkt))
                pair_lead = nl_span > 1 and ti % 2 == 0 and F_next > 0
                pair_trail = nl_span > 1 and ti % 2 == 1
                if pair_trail:
                    nt_full, n_off = span_state["nt"], span_state["F0"]
                    nt = nt_full[:, n_off:n_off + F]
                else:
                    W = F + F_next if pair_lead else F
                    nt_full = nlp.tile([128, W], ndt, tag="nt")
                    if bal3:
                        split_dma(sliver_n, nt_full, n_d[:, c0:c0 + W], W)
                    else:
                        ld_n_t.dma_start(nt_full[:], n_d[:, c0:c0 + W])
                    nt = nt_full[:, 0:F]
                    if pair_lead:
                        span_state["nt"] = nt_full
                        span_state["F0"] = F

                if stage == 0:
                    st_o.dma_start(o_d[:, c0:c0 + F], xt[:])
                    st_l.dma_start(lik_d[:, c0:c0 + F], nt[:])
                    return

                ot = iop.tile([128, F], f16, tag="ot")
                nc.vector.tensor_tensor(ot[:], xt[:], nt[:], ALU.add)
                st_o.dma_start(o_d[:, c0:c0 + F], ot[:],
                               single_packet=bool(sp_pkt))

                if pair_trail:
                    likt_full, l_off = span_state["likt"], span_state["F0"]
                    likt = likt_full[:, l_off:l_off + F]
                elif pair_lead:
                    likt_full = nlp.tile([128, F + F_next], ldt, tag="likt")
                    span_state["likt"] = likt_full
                    likt = likt_full[:, 0:F]
                else:
                    likt_full = nlp.tile([128, F], ldt, tag="likt")
                    likt = likt_full
                if variant == "sig2":
                    s1 = workp.tile([128, F], f32, tag="s1")
                    nc.scalar.activation(s1[:], ot[:], AF.Sigmoid,
                                         bias=bu_ap, scale=a_ap)
                    s2 = workp.tile([128, F], f32, tag="s2")
                    nc.scalar.activation(s2[:], ot[:], AF.Sigmoid,
                                         bias=bl_ap, scale=a_ap)
                    if lik_dtype in ("i8", "u8"):
                        df = workp.tile([128, F], f32, tag="df")
                        sub.tensor_tensor(df[:], s1[:], s2[:], ALU.subtract)
                        nc.vector.tensor_scalar(likt[:], df[:],
                                                LIK_SCALES[lik_dtype],
                                                None, ALU.mult)
                    else:
                        sub.tensor_tensor(likt[:], s1[:], s2[:], ALU.subtract)
                else:  # "tanh": lik = 2h*sigma'(t) = (h/2)*(1 - tanh(t/2)^2)
                    s1 = workp.tile([128, F], f32, tag="s1")
                    nc.scalar.activation(s1[:], ot[:], AF.Tanh,
                                         bias=d2_ap, scale=a2_ap)
                    s2 = workp.tile([128, F], f32, tag="s2")
                    nc.gpsimd.tensor_tensor(s2[:], s1[:], s1[:], ALU.mult)
                    nc.vector.tensor_scalar(likt[:], s2[:], nh2_ap, h2_ap,
                                            ALU.mult, ALU.add)
                if pair_lead:
                    pass  # store issued by the trailing tile of the pair
                elif pair_trail:
                    W = l_off + F
                    c0p = c0 - l_off
                    if bal3:
                        split_dma(sliver_l, lik_d[:, c0p:c0p + W],
                                  likt_full, W)
                    else:
                        st_l_t.dma_start(lik_d[:, c0p:c0p + W], likt_full[:])
                elif bal3:
                    split_dma(sliver_l, lik_d[:, c0:c0 + F], likt, F)
                else:
                    st_l_t.dma_start(lik_d[:, c0:c0 + F], likt[:])

            widths = [tile_f] * n_full + ([tail_f] if tail_f else [])
            for _ in range(reps):
                c0 = 0
                for j, F in enumerate(widths):
                    F_next = widths[j + 1] if j + 1 < len(widths) else 0
                    do_tile(c0, F, F_next)
                    c0 += F
                tile_idx[0] += len(widths) % 2  # keep pair phase aligned

    nc.compile()
    return nc


def _get_nc():
    if "nc" not in _CACHE:
        _CACHE["nc"] = _build_bass()
    return _CACHE["nc"]


def _make_prm(inputs):
    a64, d64 = _collapse_affine(inputs)
    h64 = 0.5 * a64
    prm = np.zeros((128, 8), dtype=np.float32)
    idx = np.arange(128) % C
    prm[:, 0] = a64[idx]
    prm[:, 1] = (d64 + h64)[idx]
    prm[:, 2] = (d64 - h64)[idx]
    prm[:, 3] = (0.5 * a64)[idx]
    prm[:, 4] = (0.5 * d64)[idx]
    prm[:, 5] = (0.5 * h64)[idx]
    prm[:, 6] = (-0.5 * h64)[idx]
    return prm


def _make_in_maps(inputs, n_dtype=None, cols_pad=None, **_ignored):
    """Shard + pack: per-core [62500,64] -> transposed [128, 31250] fp16."""
    if n_dtype is None:
        n_dtype = N_DTYPE
    if cols_pad is None:
        cols_pad = COLS_PAD
    if n_dtype == "f8":
        import ml_dtypes
        ndt = ml_dtypes.float8_e4m3fn
    else:
        ndt = np.float16
    x = np.asarray(inputs["inputs"], dtype=np.float32)
    nz = np.asarray(inputs["noise"], dtype=np.float32)
    x2 = x.reshape(N_CORES, COLS, 128).transpose(0, 2, 1).astype(np.float16)
    n2 = nz.reshape(N_CORES, COLS, 128).transpose(0, 2, 1).astype(ndt)
    if cols_pad != COLS:
        xp = np.zeros((N_CORES, 128, cols_pad), dtype=np.float16)
        xp[:, :, :COLS] = x2
        np_ = np.zeros((N_CORES, 128, cols_pad), dtype=ndt)
        np_[:, :, :COLS] = n2
        x2, n2 = xp, np_
    prm = _make_prm(inputs)
    return [{"x": x2[i], "n": n2[i], "prm": prm} for i in range(N_CORES)]


def _reference_numpy(inputs):
    """Faithful float32 numpy fallback for the general (f != 0) case."""
    x = np.asarray(inputs["inputs"], dtype=np.float32)
    nz = np.asarray(inputs["noise"], dtype=np.float32)
    o = x + nz
    xt = o.T[:, None, :]  # (C, 1, N)

    def softplus32(v):
        v = v.astype(np.float32)
        return (np.log1p(np.exp(-np.abs(v))) + np.maximum(v, 0)).astype(np.float32)

    def logits_cum(z):
        logits = z.astype(np.float32)
        for i in range(4):
            W = softplus32(np.asarray(inputs[f"m{i}"]))
            b = np.asarray(inputs[f"b{i}"], dtype=np.float32)
            f = np.asarray(inputs[f"f{i}"], dtype=np.float32)
            logits = np.einsum("cij,cjn->cin", W, logits).astype(np.float32) + b
            logits = logits + np.tanh(f) * np.tanh(logits)
        return logits.astype(np.float32)

    lower = logits_cum(xt - np.float32(0.5))
    upper = logits_cum(xt + np.float32(0.5))
    sign = -np.sign(lower + upper)

    def sig(v):
        return (1.0 / (1.0 + np.exp(-v.astype(np.float64)))).astype(np.float32)

    lik = np.abs(sig(sign * upper) - sig(sign * lower))
    lik = lik.reshape(C, -1).T
    lik = np.maximum(lik, np.float32(1e-9))
    return o, lik


def kernel(**inputs):
    x = np.asarray(inputs["inputs"], dtype=np.float32)

    f_zero = all(np.all(np.asarray(inputs[f"f{i}"]) == 0) for i in range(4))
    if x.shape != (N_TOTAL, C) or not f_zero:
        return _reference_numpy(inputs)

    in_maps = _make_in_maps(inputs)
    res = None
    for attempt in range(2):
        try:
            from concourse.bass_utils import run_bass_kernel_spmd
            nc = _get_nc()
            res = run_bass_kernel_spmd(nc, in_maps,
                                       core_ids=list(range(N_CORES)))
            break
        except Exception:
            _CACHE.pop("nc", None)  # rebuild on retry
            if attempt == 1:
                # device unusable -- return the faithful host computation
                return _reference_numpy(inputs)
    _CACHE["last_results"] = res

    o2 = np.stack([np.asarray(r["o"])[:, :COLS] for r in res.results])  # [8,128,COLS]
    l2 = np.stack([np.asarray(r["lik"])[:, :COLS] for r in res.results])
    o = o2.transpose(0, 2, 1).reshape(N_TOTAL, C).astype(np.float32)
    lik = l2.transpose(0, 2, 1).reshape(N_TOTAL, C).astype(np.float32)
    if LIK_DTYPE in ("i8", "u8"):
        lik *= np.float32(1.0 / LIK_SCALES[LIK_DTYPE])
    return o, lik
